# revision 25
# baseline (speedup 1.0000x reference)
"""Trainium2 Bass kernel for the merged multi-adapter LoRA layer.

Math (all fp32 reference):
    t[n,b,j,d]  = sum_m x[b,j,m] * lora_A[n,d,m]
    out[n,b,j,k] = sum_d t[n,b,j,d] * lora_B[n,k,d]

Shapes: x (4,2048,4096), lora_A (4,16,4096), lora_B (4,4096,16)
        out (4,4,2048,4096)

Sharding: data-parallel over flattened tokens (b*j = 8192 -> 1024/core on
8 cores); the tiny LoRA params are replicated.

Per-core HBM traffic: 8 MiB x (f16 in) + 32 MiB out (f16, widened on host)
+ ~2 MiB params  ->  ~117 us at 358 GB/s.

The PE HAM clock-gate throttles the PE to 1.2 GHz for most of the run, so
the whole kernel runs in 32x128 row-tiled PE mode (never switching mode,
so the PE never drains) and keeps the PE off the critical path:

  - mm2: the four adapters' D=16 contractions execute CONCURRENTLY on
    four 32-row PE tiles (adapter n reads t/B from SBUF partitions
    32n..32n+15 and writes its own PSUM bank) -- a group of four 512-wide
    matmuls costs ~one matmul's cycles (~131k -> ~35k PE cycles).
  - mm1: each m-tile's 128-deep contraction is split into four 32-deep
    quarter-contractions running concurrently on the four row tiles,
    accumulating four partial t tensors in four PSUM banks; a
    copy + 3 chained tensor_tensor adds on Vector reduce them to t f16.
    mm1 groups weave between mm2 groups with no mode switch.
  - PSUM: 4 x [128,512] mm2 tiles + 4 x [128,256] mm1 partials = 8 banks.
  - f32->f16 evacuation: 512-wide copies alternating Vector/Scalar.
  - x arrives pre-transposed/packed as [chunk, half, 128, 8, 512] f16,
    one 1 MiB DMA per half-chunk split across the Scalar/Sync trigger
    queues; scratch-tile warm-up matmul groups bridge the initial load.
  - stores: contiguous 0.5 MiB half-strips, first one ~6 matmul groups
    into the run.
"""

import numpy as np

import concourse.bacc as bacc
import concourse.bass as bass
import concourse.mybir as mybir
import concourse.tile as tile
from concourse import bass_utils
from concourse.bass import ds, ts

F32 = mybir.dt.float32
F16 = mybir.dt.float16

N_CORES = 8
B, J, M = 4, 2048, 4096
N, D, K = 4, 16, 4096
TOK = B * J                  # 8192 flattened tokens
TPC = TOK // N_CORES         # 1024 tokens per core
CH = 256                     # token chunk (mm1 granularity)
NCH = TPC // CH              # 4
N_MT = M // 128              # 32 m-tiles
NPAIR = N_MT // 2            # 16 packed m-tile pairs
NPH = NPAIR // 2             # pairs per half-chunk DMA (8)
KT = 512                     # mm2 matmul free width
NKG = K // KT                # 8 column groups
ADP = 32                     # partition stride per adapter / PE row tile
NSTRIP = CH // 128           # 128-token strips per chunk (2)
WARMUP = 12                  # scratch matmuls to un-throttle the PE HAM
ADD = mybir.AluOpType.add


def build_program():
    nc = bacc.Bacc("TRN2")

    xs = nc.dram_tensor(
        "xs", [NCH, 2, 128, NPH, 2 * CH], F16, kind="ExternalInput"
    ).ap()
    a_p = nc.dram_tensor("a_p", [128, N_MT, 128], F16, kind="ExternalInput").ap()
    b_p = nc.dram_tensor("b_p", [128, K], F16, kind="ExternalInput").ap()
    o = nc.dram_tensor("o", [N, TPC, K], F16, kind="ExternalOutput").ap()

    with tile.TileContext(nc) as tc:
        with (
            tc.tile_pool(name="apool", bufs=1) as apool,
            tc.tile_pool(name="bpool", bufs=1) as bpool,
            tc.tile_pool(name="spool", bufs=1) as spool,
            tc.tile_pool(name="xpool", bufs=2 * NCH) as xpool,
            tc.tile_pool(name="tpool", bufs=2) as tpool,
            tc.tile_pool(name="rpool", bufs=2) as rpool,
            tc.tile_pool(name="opool", bufs=13) as opool,
            tc.tile_pool(name="tps", bufs=4, space="PSUM") as tps_pool,
            tc.tile_pool(name="ops", bufs=4, space="PSUM") as ops_pool,
        ):
            xsb = {}
            for c in range(NCH):
                for h in range(2):
                    xsb[(c, h)] = xpool.tile([128, NPH, 2 * CH], F16, tag="x", name="x")
            a_sb = apool.tile([128, N_MT, 128], F16, tag="a")
            b_sb = bpool.tile([128, K], F16, tag="b")

            # the two halves of each chunk load in parallel on the two
            # hardware-DGE trigger queues (Scalar and Sync)
            nc.scalar.dma_start(xsb[(0, 0)][:], xs[0, 0])
            nc.sync.dma_start(a_sb[:], a_p[:])
            nc.sync.dma_start(xsb[(0, 1)][:], xs[0, 1])
            nc.scalar.dma_start(xsb[(1, 0)][:], xs[1, 0])
            nc.sync.dma_start(b_sb[:], b_p[:])
            nc.sync.dma_start(xsb[(1, 1)][:], xs[1, 1])
            nc.scalar.dma_start(xsb[(2, 0)][:], xs[2, 0])
            nc.sync.dma_start(xsb[(2, 1)][:], xs[2, 1])
            nc.scalar.dma_start(xsb[(3, 0)][:], xs[3, 0])
            nc.scalar.dma_start(xsb[(3, 1)][:], xs[3, 1])

            scr = spool.tile([128, KT], F16, tag="s", name="scr")
            nc.vector.memset(scr[:], 0.0)

            # HAM warm-up on the dependency-free scratch tile while the
            # first x chunk streams in (32x128 mode like everything else)
            for _ in range(WARMUP):
                w_ps = tps_pool.tile([128, CH], F32, tag="tps", name="wps")
                nc.tensor.matmul(
                    w_ps[:], lhsT=scr[ds(0, ADP), ds(0, 128)],
                    rhs=scr[ds(0, ADP), ds(0, CH)],
                    start=True, stop=True,
                    tile_position=(0, 0), skip_group_check=True,
                )

            def mm1_group(c, mt, parts):
                # one m-tile: 4 concurrent 32-deep quarter contractions
                for r in range(4):
                    nc.tensor.matmul(
                        parts[r][:],
                        lhsT=a_sb[ds(ADP * r, ADP), mt, :],
                        rhs=xsb[(c, mt // 16)][ds(ADP * r, ADP),
                                               (mt // 2) % NPH,
                                               ds((mt % 2) * CH, CH)],
                        start=(mt == 0),
                        stop=(mt == N_MT - 1),
                        tile_position=(ADP * r, 0),
                        skip_group_check=True,
                    )

            def mm1_parts():
                return [
                    tps_pool.tile([128, CH], F32, tag="tps", name="tps")
                    for _ in range(4)
                ]

            def mm1_reduce(parts):
                # t = p0 + p1 + p2 + p3 (one PSUM operand per op)
                red = rpool.tile([128, CH], F32, tag="r", name="red")
                nc.vector.tensor_copy(red[:], parts[0][:])
                nc.vector.tensor_add(red[:], red[:], parts[1][:])
                nc.vector.tensor_add(red[:], red[:], parts[2][:])
                t_sb = tpool.tile([128, CH], F16, tag="t", name="t")
                nc.vector.tensor_add(t_sb[:], red[:], parts[3][:])
                return t_sb

            # chunk 0's mm1 runs up front (woven against its x DMAs)
            parts = mm1_parts()
            for mt in range(N_MT):
                mm1_group(0, mt, parts)
            t_sb_next = mm1_reduce(parts)

            evac = 0
            for c in range(NCH):
                t_sb = t_sb_next
                if c + 1 < NCH:
                    parts = mm1_parts()
                    # chunk 0: weave into the back 10 mm2 groups only (its
                    # x lands mid-chunk); later chunks: spread over all 16
                    weave = {}
                    if c == 0:
                        splits = np.array_split(np.arange(N_MT), 10)
                        for g in range(6, 16):
                            weave[g] = [int(mt) for mt in splits[g - 6]]
                    else:
                        for g in range(16):
                            weave[g] = [2 * g, 2 * g + 1]

                for s in range(NSTRIP):
                    osb = [
                        opool.tile([128, K], F16, tag="o", name="osb")
                        for _ in range(N)
                    ]
                    for kg in range(NKG):
                        g = s * NKG + kg
                        o_ps = [
                            ops_pool.tile([128, KT], F32, tag="ops", name="ops")
                            for _ in range(N)
                        ]
                        # 4 adjacent matmuls on the 4 row tiles (4 distinct
                        # PSUM banks) stream concurrently
                        for n in range(N):
                            nc.tensor.matmul(
                                o_ps[n][:],
                                lhsT=t_sb[ds(ADP * n, D), ts(s, 128)],
                                rhs=b_sb[ds(ADP * n, D), ts(kg, KT)],
                                start=True,
                                stop=True,
                                tile_position=(ADP * n, 0),
                                skip_group_check=True,
                            )
                        for n in range(N):
                            if (kg + n) % 2 == 0:
                                nc.vector.tensor_copy(osb[n][:, ts(kg, KT)], o_ps[n][:])
                            else:
                                nc.scalar.copy(osb[n][:, ts(kg, KT)], o_ps[n][:])

                        # weave the next chunk's mm1 between mm2 groups
                        if c + 1 < NCH:
                            for mt in weave.get(g, []):
                                mm1_group(c + 1, mt, parts)
                                if mt == N_MT - 1:
                                    t_sb_next = mm1_reduce(parts)

                        # chunk 0: early 0.5 MiB half-stores get the wire
                        # going; later chunks: full 1 MiB stores (8 KiB
                        # rows -> better wire efficiency)
                        if c == 0 and kg % 4 == 3:
                            h = kg // 4
                            for n in range(N):
                                nc.sync.dma_start(
                                    o[n, ds(c * CH + s * 128, 128),
                                      ds(h * 4 * KT, 4 * KT)],
                                    osb[n][:, ds(h * 4 * KT, 4 * KT)],
                                )
                        elif c > 0 and kg == NKG - 1:
                            for n in range(N):
                                nc.sync.dma_start(
                                    o[n, ds(c * CH + s * 128, 128), :],
                                    osb[n][:],
                                )

    nc.compile()
    return nc


_NC_CACHE = []


def _get_nc():
    if not _NC_CACHE:
        _NC_CACHE.append(build_program())
    return _NC_CACHE[0]


def prepare_inputs(x, lora_A, lora_B):
    x = np.ascontiguousarray(np.asarray(x, dtype=np.float32)).astype(np.float16)
    lora_A = np.asarray(lora_A, dtype=np.float32)
    lora_B = np.asarray(lora_B, dtype=np.float32)

    xf = x.reshape(TOK, M)

    # a_t[m, 32n+d] = lora_A[n, d, m]; packed to [p, mt, c] so each SBUF
    # partition reads one contiguous row.
    a_t = np.zeros((M, 128), dtype=np.float32)
    for n in range(N):
        a_t[:, ADP * n : ADP * n + D] = lora_A[n].T
    a_pack = np.ascontiguousarray(
        a_t.reshape(N_MT, 128, 128).transpose(1, 0, 2)
    ).astype(np.float16)

    # b_pad[32n+d, k] = lora_B[n, k, d]
    b_pad = np.zeros((128, K), dtype=np.float16)
    for n in range(N):
        b_pad[ADP * n : ADP * n + D, :] = lora_B[n].T

    in_maps = []
    for c in range(N_CORES):
        # xp[chunk, half, p, pq, sub*CH + t] = x^T[(2*(8h+pq)+sub)*128 + p,
        #                                          chunk*CH + t]
        xT = xf[c * TPC : (c + 1) * TPC].T                  # [M, TPC]
        xr = xT.reshape(2, NPH, 2, 128, NCH, CH)            # [h, pq, sub, p, ch, t]
        xp = np.ascontiguousarray(xr.transpose(4, 0, 3, 1, 2, 5)).reshape(
            NCH, 2, 128, NPH, 2 * CH
        )
        in_maps.append({"xs": xp, "a_p": a_pack, "b_p": b_pad})
    return in_maps


def run(x, lora_A, lora_B, trace=False, **spmd_kwargs):
    nc = _get_nc()
    in_maps = prepare_inputs(x, lora_A, lora_B)
    res = bass_utils.run_bass_kernel_spmd(
        nc, in_maps, list(range(N_CORES)), trace=trace, **spmd_kwargs
    )
    o_full = np.concatenate(
        [res.results[c]["o"].astype(np.float32) for c in range(N_CORES)], axis=1
    )
    return o_full.reshape(N, B, J, K), res


def kernel(x, lora_A, lora_B):
    out, _ = run(x, lora_A, lora_B)
    return out
